# revision 1
# baseline (speedup 1.0000x reference)
"""Bipartite GNN message-passing kernel for Trainium2 (8 NeuronCores).

Strategy:
  - dst is sorted -> shard queries (50000/8=6250 per core); each core gets a
    contiguous edge range. No cross-core reduction needed.
  - Per core, queries are processed in blocks of 124. Per block, edges are
    split into two halves by src (<25000 / >=25000) so gather indices fit in
    int16, padded to a uniform NS*128 subtile structure.
  - A packed per-obs table (v in bf16, pos_obs + |o|^2 term in fp32; 512B
    rows) is built on device, then rows are fetched per edge with dma_gather.
  - dst-side features are expanded per edge with mask matmuls (is_equal of
    iota vs dst_rel); segment softmax-sum is a mask^T matmul into PSUM.
  - Softmax runs without max-subtraction: logits <= ~2 and a +60 shift keeps
    every denominator >> 1e-16 while exp stays in fp32 range (dist2 <= 3 ->
    logits >= -22 after shift).
"""

import math
import numpy as np

N_O = 50000
N_Q = 50000
E_TOT = 1_600_000
LATENT = 128
HEADS = 4
HEAD_DIM = 32
NCORES = 8
QPC = N_Q // NCORES          # queries per core
QB = 124                     # queries per block (124 + 4 obs-feature rows = 128)
NBLK = math.ceil(QPC / QB)   # 51
HALF = 25000                 # src split point for int16 indices
HROWS = 25088                # 196*128, padded rows per half-table
C_SHIFT = 60.0

_PROG_CACHE = {}


def _build_program(NS, inv_sig2, has_b1b2v):
    import concourse.bacc as bacc
    import concourse.bass as bass
    import concourse.mybir as mybir
    import concourse.tile as tile
    from contextlib import ExitStack

    dt = mybir.dt
    f32, bf16, i16 = dt.float32, dt.bfloat16, dt.int16
    AF = mybir.ActivationFunctionType
    OP = mybir.AluOpType
    NSP = NS * 128  # edges per (block, half)

    nc = bacc.Bacc("TRN2", target_bir_lowering=False, debug=False)

    # ---- DRAM tensors (per-core inputs) ----
    hA = nc.dram_tensor("hA", [HROWS, LATENT], f32, kind="ExternalInput")
    hB = nc.dram_tensor("hB", [HROWS, LATENT], f32, kind="ExternalInput")
    pA = nc.dram_tensor("pA", [HROWS, 3], f32, kind="ExternalInput")
    pB = nc.dram_tensor("pB", [HROWS, 3], f32, kind="ExternalInput")
    posq_blk = nc.dram_tensor("posq_blk", [128, NBLK * 4], f32, kind="ExternalInput")
    idxw = nc.dram_tensor("idxw", [NBLK * 2 * 128, NS * 8], i16, kind="ExternalInput")
    dstrel = nc.dram_tensor("dstrel", [NBLK * 2 * 128, NS], bf16, kind="ExternalInput")
    w1qcb1 = nc.dram_tensor("w1qcb1", [4, 128], f32, kind="ExternalInput")
    w1oc = nc.dram_tensor("w1oc", [4, 128], bf16, kind="ExternalInput")
    w1ocf = nc.dram_tensor("w1ocf", [4, 128], f32, kind="ExternalInput")
    sel124 = nc.dram_tensor("sel124", [4, 128], f32, kind="ExternalInput")
    w2 = nc.dram_tensor("w2", [128, 4], bf16, kind="ExternalInput")
    wv = nc.dram_tensor("wv", [128, 128], bf16, kind="ExternalInput")
    b2rep = nc.dram_tensor("b2rep", [128, 4], f32, kind="ExternalInput")
    bvrep = nc.dram_tensor("bvrep", [128, 128], f32, kind="ExternalInput")
    iota_in = nc.dram_tensor("iota_in", [128, 128], bf16, kind="ExternalInput")
    idf32 = nc.dram_tensor("idf32", [128, 128], f32, kind="ExternalInput")
    idbf = nc.dram_tensor("idbf", [128, 128], bf16, kind="ExternalInput")

    GA = nc.dram_tensor("GA", [HROWS, 128], f32)   # packed table half A
    GB = nc.dram_tensor("GB", [HROWS, 128], f32)   # packed table half B
    out_d = nc.dram_tensor("out", [NBLK * QB, 128], f32, kind="ExternalOutput")

    NTIL = HROWS // 128  # 196 tiles per half

    with tile.TileContext(nc) as tc, ExitStack() as ctx:
        cpool = ctx.enter_context(tc.tile_pool(name="consts", bufs=1))
        # resident tables built in the prologue
        aw1_sb = cpool.tile([128, NBLK * 128], bf16, tag="aw1")
        pqx_sb = cpool.tile([128, NBLK * 8], bf16, tag="pqx")

        # load constants
        iota_sb = cpool.tile([128, 128], bf16, tag="iota")
        nc.sync.dma_start(iota_sb[:], iota_in[:])
        idf_sb = cpool.tile([128, 128], f32, tag="idf")
        nc.sync.dma_start(idf_sb[:], idf32[:])
        idb_sb = cpool.tile([128, 128], bf16, tag="idb")
        nc.sync.dma_start(idb_sb[:], idbf[:])
        w1qc_sb = cpool.tile([4, 128], f32, tag="w1qc")
        nc.sync.dma_start(w1qc_sb[:], w1qcb1[:])
        w1oc_sb = cpool.tile([4, 128], bf16, tag="w1oc")
        nc.sync.dma_start(w1oc_sb[:], w1oc[:])
        w1ocf_sb = cpool.tile([4, 128], f32, tag="w1ocf")
        nc.sync.dma_start(w1ocf_sb[:], w1ocf[:])
        sel_sb = cpool.tile([4, 128], f32, tag="sel")
        nc.sync.dma_start(sel_sb[:], sel124[:])
        w2_sb = cpool.tile([128, 4], bf16, tag="w2")
        nc.sync.dma_start(w2_sb[:], w2[:])
        wv_sb = cpool.tile([128, 128], bf16, tag="wv")
        nc.sync.dma_start(wv_sb[:], wv[:])
        b2_sb = cpool.tile([128, 4], f32, tag="b2")
        nc.sync.dma_start(b2_sb[:], b2rep[:])
        bv_sb = cpool.tile([128, 128], f32, tag="bv")
        nc.sync.dma_start(bv_sb[:], bvrep[:])
        pq_sb = cpool.tile([128, NBLK * 4], f32, tag="pq")
        nc.sync.dma_start(pq_sb[:], posq_blk[:])

        # ---------- prologue A: packed obs table (v | pos | co) ----------
        with tc.tile_pool(name="gb_sb", bufs=3) as gp, \
             tc.tile_pool(name="gb_ps", bufs=2, space="PSUM") as gpp, \
             tc.tile_pool(name="gb_ps2", bufs=2, space="PSUM") as gpp2:
            for t_half, (hsrc, psrc, gdst) in enumerate(((hA, pA, GA), (hB, pB, GB))):
                for t in range(NTIL):
                    hbf = gp.tile([128, 128], bf16, tag="hbf")
                    nc.gpsimd.dma_start(hbf[:], hsrc[t * 128:(t + 1) * 128, :])
                    htp = gpp2.tile([128, 128], bf16, tag="htp", space="PSUM")
                    nc.tensor.transpose(out=htp[:], in_=hbf[:], identity=idb_sb[:])
                    hT = gp.tile([128, 128], bf16, tag="hT")
                    nc.scalar.copy(out=hT[:], in_=htp[:])
                    vps = gpp.tile([128, 128], f32, tag="vps", space="PSUM")
                    nc.tensor.matmul(out=vps[:], lhsT=hT[:], rhs=wv_sb[:],
                                     start=True, stop=True)
                    gt = gp.tile([128, 128], f32, tag="gt")
                    if has_b1b2v:
                        vsum = gp.tile([128, 128], f32, tag="vsum")
                        nc.vector.tensor_tensor(out=vsum[:], in0=vps[:],
                                                in1=bv_sb[:], op=OP.add)
                        nc.vector.tensor_copy(
                            out=gt[:, 0:64].bitcast(bf16), in_=vsum[:])
                    else:
                        nc.vector.tensor_copy(
                            out=gt[:, 0:64].bitcast(bf16), in_=vps[:])
                    nc.sync.dma_start(gt[:, 64:67],
                                      psrc[t * 128:(t + 1) * 128, :])
                    nc.gpsimd.memset(gt[:, 67:128], 0.0)
                    nc.gpsimd.dma_start(gdst[t * 128:(t + 1) * 128, :], gt[:])

            # ---------- prologue B: A-table + posqext ----------
            for b in range(NBLK):
                pq4 = gp.tile([128, 4], f32, tag="pq4")
                nc.vector.tensor_copy(out=pq4[:], in_=pq_sb[:, b * 4:b * 4 + 4])
                tps = gpp.tile([128, 128], f32, tag="tps", space="PSUM")
                nc.tensor.transpose(out=tps[0:4, :], in_=pq4[:], identity=idf_sb[:])
                pqT = gp.tile([4, 128], f32, tag="pqT")
                nc.scalar.copy(out=pqT[:], in_=tps[0:4, :])
                aps = gpp2.tile([128, 128], f32, tag="aps", space="PSUM")
                nc.tensor.matmul(out=aps[:], lhsT=pqT[:], rhs=w1qc_sb[:],
                                 start=True, stop=False)
                nc.tensor.matmul(out=aps[:], lhsT=sel_sb[:], rhs=w1ocf_sb[:],
                                 start=False, stop=True)
                nc.scalar.copy(out=aw1_sb[:, b * 128:(b + 1) * 128], in_=aps[:])

            # posqext: [sh1*3, sh2*3, cqh1, cqh2] per query, bf16
            s_all = gp.tile([128, NBLK * 3], f32, tag="s_all")
            nc.scalar.activation(
                out=s_all[:],
                in_=pq_sb[:].rearrange("p (b i) -> p b i", i=4)[:, :, 0:3],
                func=AF.Copy, bias=0.0, scale=float(inv_sig2))
            sh1 = gp.tile([128, NBLK * 3], bf16, tag="sh1")
            nc.vector.tensor_copy(out=sh1[:], in_=s_all[:])
            sh1f = gp.tile([128, NBLK * 3], f32, tag="sh1f")
            nc.vector.tensor_copy(out=sh1f[:], in_=sh1[:])
            sh2 = gp.tile([128, NBLK * 3], bf16, tag="sh2")
            nc.vector.tensor_tensor(out=sh2[:], in0=s_all[:], in1=sh1f[:],
                                    op=OP.subtract)
            q2 = gp.tile([128, NBLK * 3], f32, tag="q2")
            pqv = pq_sb[:].rearrange("p (b i) -> p b i", i=4)[:, :, 0:3]
            nc.vector.tensor_tensor(out=q2[:], in0=pqv, in1=pqv, op=OP.mult)
            cq = gp.tile([128, NBLK], f32, tag="cq")
            nc.vector.tensor_reduce(
                out=cq[:], in_=q2[:].rearrange("p (b i) -> p b i", i=3),
                axis=mybir.AxisListType.X, op=OP.add)
            nc.scalar.activation(out=cq[:], in_=cq[:], func=AF.Copy,
                                 bias=0.0, scale=float(-inv_sig2 / 2.0))
            cqh1 = gp.tile([128, NBLK], bf16, tag="cqh1")
            nc.vector.tensor_copy(out=cqh1[:], in_=cq[:])
            cqh1f = gp.tile([128, NBLK], f32, tag="cqh1f")
            nc.vector.tensor_copy(out=cqh1f[:], in_=cqh1[:])
            cqh2 = gp.tile([128, NBLK], bf16, tag="cqh2")
            nc.vector.tensor_tensor(out=cqh2[:], in0=cq[:], in1=cqh1f[:],
                                    op=OP.subtract)
            pqxv = pqx_sb[:].rearrange("p (b i) -> p b i", i=8)
            nc.vector.tensor_copy(
                out=pqxv[:, :, 0:3],
                in_=sh1[:].rearrange("p (b i) -> p b i", i=3))
            nc.vector.tensor_copy(
                out=pqxv[:, :, 3:6],
                in_=sh2[:].rearrange("p (b i) -> p b i", i=3))
            nc.vector.tensor_copy(out=pqxv[:, :, 6:7],
                                  in_=cqh1[:].unsqueeze(2))
            nc.vector.tensor_copy(out=pqxv[:, :, 7:8],
                                  in_=cqh2[:].unsqueeze(2))
            # obs-feature rows 124-127 are zero by construction (posq_blk
            # host rows 124-127 are zero and cq no longer carries C_SHIFT)

        # ---------- main edge loop ----------
        mpool = ctx.enter_context(tc.tile_pool(name="main", bufs=3))
        ppool = ctx.enter_context(tc.tile_pool(name="mpsA", bufs=2, space="PSUM"))
        ppool2 = ctx.enter_context(tc.tile_pool(name="mpsB", bufs=2, space="PSUM"))
        ppoolT = ctx.enter_context(tc.tile_pool(name="mpsT", bufs=2, space="PSUM"))
        opool = ctx.enter_context(tc.tile_pool(name="mpsO", bufs=2, space="PSUM"))

        for b in range(NBLK):
            pout = opool.tile([128, 132], f32, tag="pout", space="PSUM")
            for half in range(2):
                row0 = (b * 2 + half) * 128
                gsrc = GA if half == 0 else GB
                idx_sb = mpool.tile([128, NS * 8], i16, tag="idx")
                nc.sync.dma_start(idx_sb[:], idxw[row0:row0 + 128, :])
                drt = mpool.tile([128, NS], bf16, tag="drt")
                nc.sync.dma_start(drt[:], dstrel[row0:row0 + 128, :])
                gt = mpool.tile([128, NS * 128], f32, tag="gt")
                # HW SWDGE ring holds 1024 descriptors (raising
                # dynamic_dma_scratch_size does NOT lift the HW limit —
                # >1024-idx gathers hang the device). Chunk at 1024.
                gv = gt[:].rearrange("p (n k) -> p n k", k=128)
                for c0 in range(0, NS, 8):
                    c1 = min(c0 + 8, NS)
                    n_c = (c1 - c0) * 128
                    nc.gpsimd.dma_gather(
                        out_ap=gv[:, c0:c1, :],
                        in_ap=gsrc[:],
                        idxs_ap=idx_sb[:, c0 * 8:c0 * 8 + n_c // 16],
                        num_idxs=n_c,
                        num_idxs_reg=n_c,
                        elem_size=128,
                    )
                gtb = gt[:].bitcast(bf16)  # [128, NS*256]

                mext = mpool.tile([128, NS * 128], bf16, tag="mext")
                mev = mext[:].rearrange("p (n k) -> p n k", k=128)
                nc.vector.tensor_tensor(
                    out=mev[:, :, 0:124],
                    in0=iota_sb[:, 0:124].unsqueeze(1).broadcast_to([128, NS, 124]),
                    in1=drt[:].unsqueeze(2).broadcast_to([128, NS, 124]),
                    op=OP.is_equal)
                nc.vector.tensor_copy(
                    out=mev[:, :, 124:128],
                    in_=gt[:].rearrange("p (n k) -> p n k", k=128)[:, :, 64:68])

                stack = mpool.tile([128, NS * 128], bf16, tag="stack")
                hid = mpool.tile([128, NS * 128], bf16, tag="hid")
                pmisc = ppool2.tile([128, NS * 12], f32, tag="pmisc", space="PSUM")
                qdv = pmisc[:].rearrange("p (n k) -> p n k", k=12)

                for j in range(NS):
                    tps = ppoolT.tile([128, 128], bf16, tag="tps", space="PSUM")
                    nc.tensor.transpose(
                        out=tps[:], in_=mext[:, j * 128:(j + 1) * 128],
                        identity=idb_sb[:])
                    nc.scalar.copy(out=stack[:, j * 128:(j + 1) * 128], in_=tps[:])
                    if j % 4 == 0:
                        phid_cur = ppool.tile([128, 512], f32, tag="phid",
                                              space="PSUM")
                    nc.tensor.matmul(
                        out=phid_cur[:, (j % 4) * 128:((j % 4) + 1) * 128],
                        lhsT=aw1_sb[:, b * 128:(b + 1) * 128],
                        rhs=stack[:, j * 128:(j + 1) * 128],
                        start=True, stop=True)
                    nc.tensor.matmul(
                        out=qdv[:, j, 0:8],
                        lhsT=stack[:, j * 128:(j + 1) * 128],
                        rhs=pqx_sb[:, b * 8:b * 8 + 8],
                        start=True, stop=True)
                    if j % 4 == 3 or j == NS - 1:
                        lo = (j // 4) * 512
                        w = (j % 4 + 1) * 128
                        nc.scalar.activation(
                            out=hid[:, lo:lo + w], in_=phid_cur[:, 0:w],
                            func=AF.Relu, bias=0.0, scale=1.0)
                for j in range(NS):
                    nc.tensor.matmul(
                        out=qdv[:, j, 8:12],
                        lhsT=hid[:, j * 128:(j + 1) * 128],
                        rhs=w2_sb[:],
                        start=True, stop=True)

                # d = sum_i o_i*(sh1_i + sh2_i - inv2/2*o_i) + cqh1 + cqh2
                ov = gt[:].rearrange("p (n k) -> p n k", k=128)[:, :, 64:67]
                ta = mpool.tile([128, NS * 3], f32, tag="ta")
                nc.vector.tensor_tensor(
                    out=ta[:].rearrange("p (n i) -> p n i", i=3),
                    in0=qdv[:, :, 0:3], in1=ov, op=OP.mult)
                tb = mpool.tile([128, NS * 3], f32, tag="tb")
                nc.vector.tensor_tensor(
                    out=tb[:].rearrange("p (n i) -> p n i", i=3),
                    in0=qdv[:, :, 3:6], in1=ov, op=OP.mult)
                tab = mpool.tile([128, NS * 3], f32, tag="tab")
                nc.vector.tensor_tensor(out=tab[:], in0=ta[:], in1=tb[:],
                                        op=OP.add)
                u = mpool.tile([128, NS * 3], f32, tag="u")
                nc.scalar.activation(
                    out=u[:].rearrange("p (n i) -> p n i", i=3), in_=ov,
                    func=AF.Copy, bias=0.0, scale=float(-inv_sig2 / 2.0))
                u2 = mpool.tile([128, NS * 3], f32, tag="u2")
                nc.vector.tensor_tensor(
                    out=u2[:].rearrange("p (n i) -> p n i", i=3),
                    in0=u[:].rearrange("p (n i) -> p n i", i=3),
                    in1=ov, op=OP.mult)
                t2 = mpool.tile([128, NS * 3], f32, tag="t2")
                nc.vector.tensor_tensor(out=t2[:], in0=tab[:], in1=u2[:],
                                        op=OP.add)
                dsum = mpool.tile([128, NS], f32, tag="dsum")
                nc.vector.tensor_reduce(
                    out=dsum[:], in_=t2[:].rearrange("p (n i) -> p n i", i=3),
                    axis=mybir.AxisListType.X, op=OP.add)
                dsh = mpool.tile([128, NS], f32, tag="dsh")
                nc.scalar.activation(out=dsh[:], in_=dsum[:], func=AF.Copy,
                                     bias=float(C_SHIFT), scale=1.0)
                d1 = mpool.tile([128, NS], f32, tag="d1")
                nc.vector.tensor_tensor(out=d1[:], in0=dsh[:],
                                        in1=qdv[:, :, 6:7].squeeze(2), op=OP.add)
                dfin = mpool.tile([128, NS], f32, tag="dfin")
                nc.vector.tensor_tensor(out=dfin[:], in0=d1[:],
                                        in1=qdv[:, :, 7:8].squeeze(2), op=OP.add)
                lst = mpool.tile([128, NS * 4], f32, tag="lst")
                nc.vector.tensor_tensor(
                    out=lst[:].rearrange("p (n h) -> p n h", h=4),
                    in0=qdv[:, :, 8:12],
                    in1=dfin[:].unsqueeze(2).broadcast_to([128, NS, 4]),
                    op=OP.add)
                if has_b1b2v:
                    nc.vector.tensor_tensor(
                        out=lst[:].rearrange("p (n h) -> p n h", h=4),
                        in0=lst[:].rearrange("p (n h) -> p n h", h=4),
                        in1=b2_sb[:].unsqueeze(1).broadcast_to([128, NS, 4]),
                        op=OP.add)
                ex = mpool.tile([128, NS * 4], bf16, tag="ex")
                nc.scalar.activation(out=ex[:], in_=lst[:], func=AF.Exp,
                                     bias=0.0, scale=1.0)

                vse = mpool.tile([128, NS * 132], bf16, tag="vse")
                vsev = vse[:].rearrange("p (n k) -> p n k", k=132)
                nc.vector.tensor_tensor(
                    out=vsev[:, :, 0:128].rearrange("p n (h w) -> p n h w", w=32),
                    in0=gtb.rearrange("p (n k) -> p n k", k=256)[:, :, 0:128]
                        .rearrange("p n (h w) -> p n h w", w=32),
                    in1=ex[:].rearrange("p (n h) -> p n h", h=4).unsqueeze(3)
                        .broadcast_to([128, NS, 4, 32]),
                    op=OP.mult)
                nc.vector.tensor_copy(
                    out=vsev[:, :, 128:132],
                    in_=ex[:].rearrange("p (n h) -> p n h", h=4))
                for j in range(NS):
                    nc.tensor.matmul(
                        out=pout[:],
                        lhsT=mext[:, j * 128:(j + 1) * 128],
                        rhs=vse[:, j * 132:(j + 1) * 132],
                        start=(half == 0 and j == 0),
                        stop=(half == 1 and j == NS - 1))

            den = mpool.tile([128, 4], f32, tag="den")
            nc.scalar.activation(out=den[:], in_=pout[:, 128:132],
                                 func=AF.Copy, bias=1e-30, scale=1.0)
            rec = mpool.tile([128, 4], f32, tag="rec")
            nc.vector.reciprocal(out=rec[:], in_=den[:])
            onorm = mpool.tile([128, 128], f32, tag="onorm")
            nc.vector.tensor_tensor(
                out=onorm[:].rearrange("p (h w) -> p h w", w=32),
                in0=pout[:, 0:128].rearrange("p (h w) -> p h w", w=32),
                in1=rec[:].unsqueeze(2).broadcast_to([128, 4, 32]),
                op=OP.mult)
            nc.sync.dma_start(out_d[b * QB:(b + 1) * QB, :], onorm[0:QB, :])

    nc.compile()
    return nc


def _host_prep(h_obs, pos_obs, pos_query, src, dst, W1, b1, W2, b2, Wv, bv,
               log_sigma):
    import ml_dtypes
    bf = ml_dtypes.bfloat16

    src = np.asarray(src).astype(np.int64)
    dst = np.asarray(dst).astype(np.int64)
    h_obs = np.asarray(h_obs, dtype=np.float32)
    pos_obs = np.asarray(pos_obs, dtype=np.float32)
    pos_query = np.asarray(pos_query, dtype=np.float32)
    W1 = np.asarray(W1, dtype=np.float32)
    W2 = np.asarray(W2, dtype=np.float32)
    Wv = np.asarray(Wv, dtype=np.float32)
    b1 = np.asarray(b1, dtype=np.float32)
    b2 = np.asarray(b2, dtype=np.float32)
    bv = np.asarray(bv, dtype=np.float32)
    sigma = np.exp(np.float32(log_sigma)) + np.float32(1e-6)
    inv_sig2 = float(1.0 / (np.float64(sigma) ** 2))

    # per-core edge partition, then per (core, block, half) lists
    core_lists = []  # [core][block][half] -> (src_half, dst_rel)
    max_ns = 1
    edge_bounds = np.searchsorted(dst, np.arange(NCORES + 1) * QPC)
    for c in range(NCORES):
        e0, e1 = edge_bounds[c], edge_bounds[c + 1]
        dl = dst[e0:e1] - c * QPC
        sl = src[e0:e1]
        blocks = []
        blk_bounds = np.searchsorted(dl, np.arange(NBLK + 1) * QB)
        for b in range(NBLK):
            be0, be1 = blk_bounds[b], blk_bounds[b + 1]
            bsrc = sl[be0:be1]
            bdr = dl[be0:be1] - b * QB
            m = bsrc < HALF
            halves = []
            for hm, off in ((m, 0), (~m, HALF)):
                s_h = (bsrc[hm] - off).astype(np.int16)
                d_h = bdr[hm].astype(np.float32)
                halves.append((s_h, d_h))
                max_ns = max(max_ns, math.ceil(len(s_h) / 128))
            blocks.append(halves)
        core_lists.append(blocks)
    NS = max_ns
    NSP = NS * 128

    in_maps = []
    iota = np.broadcast_to(np.arange(128, dtype=np.float32), (128, 128))
    ident = np.eye(128, dtype=np.float32)
    w1qcb1 = np.concatenate([W1[0:3] + W1[3:6], b1[None, :]], 0).astype(np.float32)
    w1oc = np.zeros((4, 128), np.float32)
    w1oc[0:3] = W1[6:9] - W1[0:3]
    has_b1b2v = bool(np.any(b1) or np.any(b2) or np.any(bv))

    hpadA = np.zeros((HROWS, LATENT), np.float32)
    hpadA[:HALF] = h_obs[:HALF]
    hpadB = np.zeros((HROWS, LATENT), np.float32)
    hpadB[:N_O - HALF] = h_obs[HALF:]
    ppadA = np.zeros((HROWS, 3), np.float32)
    ppadA[:HALF] = pos_obs[:HALF]
    ppadB = np.zeros((HROWS, 3), np.float32)
    ppadB[:N_O - HALF] = pos_obs[HALF:]

    for c in range(NCORES):
        idxw = np.zeros((NBLK * 2, 128, NS * 8), np.int16)
        drel = np.full((NBLK * 2, 128, NS), -1.0, np.float32)
        for b in range(NBLK):
            for half in range(2):
                s_h, d_h = core_lists[c][b][half]
                n = len(s_h)
                ip = np.zeros(NSP, np.int16)
                ip[:n] = s_h
                w = ip.reshape(NSP // 16, 16).T  # [16, NS*8]
                idxw[b * 2 + half] = np.tile(w, (8, 1))
                dp = np.full(NSP, -1.0, np.float32)
                dp[:n] = d_h
                drel[b * 2 + half] = dp.reshape(NS, 128).T
        pqb = np.zeros((128, NBLK * 4), np.float32)
        qs = pos_query[c * QPC:(c + 1) * QPC]
        for b in range(NBLK):
            lo, hi = b * QB, min((b + 1) * QB, QPC)
            pqb[:hi - lo, b * 4:b * 4 + 3] = qs[lo:hi]
            pqb[:hi - lo, b * 4 + 3] = 1.0
        in_maps.append({
            "hA": hpadA, "hB": hpadB, "pA": ppadA, "pB": ppadB,
            "posq_blk": pqb,
            "idxw": idxw.reshape(NBLK * 2 * 128, NS * 8),
            "dstrel": drel.reshape(NBLK * 2 * 128, NS).astype(bf),
            "w1qcb1": w1qcb1,
            "w1oc": w1oc.astype(bf),
            "w1ocf": w1oc,
            "sel124": np.eye(128, dtype=np.float32)[124:128],
            "w2": W2.astype(bf),
            "wv": Wv.astype(bf),
            "b2rep": np.broadcast_to(b2, (128, 4)).copy().astype(np.float32),
            "bvrep": np.broadcast_to(bv, (128, 128)).copy().astype(np.float32),
            "iota_in": iota.astype(bf),
            "idf32": ident,
            "idbf": ident.astype(bf),
        })
    return NS, inv_sig2, has_b1b2v, in_maps


def kernel(h_obs, pos_obs, pos_query, src, dst, W1, b1, W2, b2, Wv, bv,
           log_sigma, **_unused):
    import sys
    for p in ("/opt/trn_rl_repo", "/root/.axon_site/_ro/trn_rl_repo"):
        if p not in sys.path:
            sys.path.append(p)
    from concourse.bass_utils import run_bass_kernel_spmd

    NS, inv_sig2, has_b1b2v, in_maps = _host_prep(
        h_obs, pos_obs, pos_query, src, dst, W1, b1, W2, b2, Wv, bv, log_sigma)

    key = (NS, round(inv_sig2, 9), has_b1b2v)
    if key not in _PROG_CACHE:
        _PROG_CACHE[key] = _build_program(NS, inv_sig2, has_b1b2v)
    nc = _PROG_CACHE[key]

    res = run_bass_kernel_spmd(nc, in_maps, core_ids=list(range(NCORES)))
    outs = [np.asarray(r["out"])[:QPC] for r in res.results]
    return np.concatenate(outs, axis=0).astype(np.float32)


if __name__ == "__main__":
    rng = np.random.default_rng(0)
    pass



# revision 4
# speedup vs baseline: 2.7593x; 2.7593x over previous
"""Bipartite GNN message-passing kernel for Trainium2 (8 NeuronCores).

Strategy (v3):
  - dst is sorted -> shard queries (50000/8=6250 per core); each core gets a
    contiguous edge range. No cross-core reduction needed.
  - Per core, queries are processed in blocks of 124. Per block, edges are
    split into two halves by src (<25000 / >=25000) so gather indices fit in
    int16, padded to a uniform NS*128 subtile structure.
  - A packed per-obs table (v in bf16 head-interleaved, pos_obs f32,
    co = -inv2/2*|o|^2 + C_SHIFT; 512B rows) is built on device from a
    host-pretransposed bf16 h, then rows are fetched per edge with
    dma_gather.
  - Per-edge query one-hot (mext) is built with per-subtile tensor_scalar
    is_equal (4x DVE mode); transposed stack via PE transpose + batched
    PSUM->SBUF copies split across Act/DVE/Pool.
  - v is stored head-interleaved (col = w*4+h) so the attention scaling
    multiply has a packed 4-wide inner dim (2x DVE mode). Host un-interleaves
    the output.
  - Softmax runs without max-subtraction: logits <= ~2 and a +60 shift
    (folded into co) keeps every denominator >> 1e-16 while exp stays in
    fp32/bf16 range.
"""

import math
import numpy as np

N_O = 50000
N_Q = 50000
E_TOT = 1_600_000
LATENT = 128
HEADS = 4
HEAD_DIM = 32
NCORES = 8
QPC = N_Q // NCORES          # queries per core
QB = 124                     # queries per block (124 + 4 obs-feature rows = 128)
NBLK = math.ceil(QPC / QB)   # 51
HALF = 25000                 # src split point for int16 indices
HROWS = 25088                # 196*128, padded rows per half-table
NTIL = HROWS // 128          # 196 tiles per half
C_SHIFT = 60.0
TB = 16                      # table-build tiles per DMA batch

_PROG_CACHE = {}


def _build_program(NS, inv_sig2, has_b1b2v):
    import concourse.bacc as bacc
    import concourse.bass as bass
    import concourse.mybir as mybir
    import concourse.tile as tile
    from contextlib import ExitStack

    dt = mybir.dt
    f32, bf16, i16 = dt.float32, dt.bfloat16, dt.int16
    AF = mybir.ActivationFunctionType
    OP = mybir.AluOpType
    NSP = NS * 128  # edges per (block, half)
    NCH = math.ceil(NS / 4)  # 4-subtile chunks per half

    nc = bacc.Bacc("TRN2", target_bir_lowering=False, debug=False)

    # ---- DRAM tensors (per-core inputs) ----
    hTA = nc.dram_tensor("hTA", [128, HROWS], bf16, kind="ExternalInput")
    hTB = nc.dram_tensor("hTB", [128, HROWS], bf16, kind="ExternalInput")
    pcA = nc.dram_tensor("pcA", [128, NTIL * 4], f32, kind="ExternalInput")
    pcB = nc.dram_tensor("pcB", [128, NTIL * 4], f32, kind="ExternalInput")
    posq_blk = nc.dram_tensor("posq_blk", [128, NBLK * 4], f32, kind="ExternalInput")
    idxw = nc.dram_tensor("idxw", [NBLK * 2 * 128, NS * 8], i16, kind="ExternalInput")
    dstrel = nc.dram_tensor("dstrel", [NBLK * 2 * 128, NS], f32, kind="ExternalInput")
    w1qcb1 = nc.dram_tensor("w1qcb1", [4, 128], f32, kind="ExternalInput")
    w1ocf = nc.dram_tensor("w1ocf", [4, 128], f32, kind="ExternalInput")
    sel124 = nc.dram_tensor("sel124", [4, 128], f32, kind="ExternalInput")
    w2 = nc.dram_tensor("w2", [128, 4], bf16, kind="ExternalInput")
    wv = nc.dram_tensor("wv", [128, 128], bf16, kind="ExternalInput")  # interleaved
    b2rep = nc.dram_tensor("b2rep", [128, 4], f32, kind="ExternalInput")
    bvrep = nc.dram_tensor("bvrep", [128, 128], f32, kind="ExternalInput")  # interleaved
    iota_in = nc.dram_tensor("iota_in", [128, 128], bf16, kind="ExternalInput")
    idf32 = nc.dram_tensor("idf32", [128, 128], f32, kind="ExternalInput")
    idbf = nc.dram_tensor("idbf", [128, 128], bf16, kind="ExternalInput")

    GA = nc.dram_tensor("GA", [HROWS, 128], f32)   # packed table half A
    GB = nc.dram_tensor("GB", [HROWS, 128], f32)   # packed table half B
    out_d = nc.dram_tensor("out", [NBLK * QB, 128], f32, kind="ExternalOutput")

    with tile.TileContext(nc) as tc, ExitStack() as ctx:
        cpool = ctx.enter_context(tc.tile_pool(name="consts", bufs=1))
        aw1_sb = cpool.tile([128, NBLK * 128], bf16, tag="aw1")
        pqx_sb = cpool.tile([128, NBLK * 8], bf16, tag="pqx")

        iota_sb = cpool.tile([128, 128], bf16, tag="iota")
        nc.sync.dma_start(iota_sb[:], iota_in[:])
        idf_sb = cpool.tile([128, 128], f32, tag="idf")
        nc.sync.dma_start(idf_sb[:], idf32[:])
        idb_sb = cpool.tile([128, 128], bf16, tag="idb")
        nc.sync.dma_start(idb_sb[:], idbf[:])
        w1qc_sb = cpool.tile([4, 128], f32, tag="w1qc")
        nc.sync.dma_start(w1qc_sb[:], w1qcb1[:])
        w1ocf_sb = cpool.tile([4, 128], f32, tag="w1ocf")
        nc.sync.dma_start(w1ocf_sb[:], w1ocf[:])
        sel_sb = cpool.tile([4, 128], f32, tag="sel")
        nc.sync.dma_start(sel_sb[:], sel124[:])
        w2_sb = cpool.tile([128, 4], bf16, tag="w2")
        nc.sync.dma_start(w2_sb[:], w2[:])
        wv_sb = cpool.tile([128, 128], bf16, tag="wv")
        nc.sync.dma_start(wv_sb[:], wv[:])
        b2_sb = cpool.tile([128, 4], f32, tag="b2")
        nc.sync.dma_start(b2_sb[:], b2rep[:])
        bv_sb = cpool.tile([128, 128], f32, tag="bv")
        nc.sync.dma_start(bv_sb[:], bvrep[:])
        pq_sb = cpool.tile([128, NBLK * 4], f32, tag="pq")
        nc.sync.dma_start(pq_sb[:], posq_blk[:])
        pcA_sb = cpool.tile([128, NTIL * 4], f32, tag="pcA")
        nc.sync.dma_start(pcA_sb[:], pcA[:])
        pcB_sb = cpool.tile([128, NTIL * 4], f32, tag="pcB")
        nc.sync.dma_start(pcB_sb[:], pcB[:])

        # ---------- prologue A: packed obs table (v | pos | co) ----------
        nbat = math.ceil(NTIL / TB)
        with tc.tile_pool(name="gb_sb", bufs=2) as gp, \
             tc.tile_pool(name="gb_sm", bufs=2) as gsm, \
             tc.tile_pool(name="gb_ps", bufs=2, space="PSUM") as gpp, \
             tc.tile_pool(name="gb_ps2", bufs=2, space="PSUM") as gpp2:
            for t_half, (hsrc, pc_sb, gdst) in enumerate(
                    ((hTA, pcA_sb, GA), (hTB, pcB_sb, GB))):
                # co into col 4t+3 of the resident pos tile:
                # co = -inv2/2 * (o0^2+o1^2+o2^2) + C_SHIFT
                pcv = pc_sb[:].rearrange("p (t i) -> p t i", i=4)
                sq = gsm.tile([128, NTIL * 3], f32, tag="sq")
                nc.vector.tensor_tensor(
                    out=sq[:].rearrange("p (t i) -> p t i", i=3),
                    in0=pcv[:, :, 0:3], in1=pcv[:, :, 0:3], op=OP.mult)
                co = gsm.tile([128, NTIL], f32, tag="co")
                nc.vector.tensor_reduce(
                    out=co[:], in_=sq[:].rearrange("p (t i) -> p t i", i=3),
                    axis=mybir.AxisListType.X, op=OP.add)
                nc.scalar.activation(
                    out=pcv[:, :, 3:4].squeeze(2), in_=co[:], func=AF.Copy,
                    bias=float(C_SHIFT), scale=float(-inv_sig2 / 2.0))

                for bt in range(nbat):
                    t0 = bt * TB
                    t1 = min(t0 + TB, NTIL)
                    nt = t1 - t0
                    hb = gp.tile([128, TB * 128], bf16, tag="hb")
                    nc.sync.dma_start(hb[:, 0:nt * 128],
                                      hsrc[:, t0 * 128:t1 * 128])
                    stg = gp.tile([128, TB * 128], f32, tag="stg")
                    for c4 in range(math.ceil(nt / 4)):
                        k0 = c4 * 4
                        k1 = min(k0 + 4, nt)
                        vps = gpp.tile([128, 512], f32, tag="vps", space="PSUM")
                        for k in range(k0, k1):
                            nc.tensor.matmul(
                                out=vps[:, (k - k0) * 128:(k - k0 + 1) * 128],
                                lhsT=hb[:, k * 128:(k + 1) * 128],
                                rhs=wv_sb[:], start=True, stop=True)
                        w = (k1 - k0) * 128
                        if has_b1b2v:
                            vsum = gsm.tile([128, 512], f32, tag="vsum")
                            nc.vector.tensor_tensor(
                                out=vsum[0:128, 0:w].rearrange(
                                    "p (t k) -> p t k", k=128),
                                in0=vps[:, 0:w].rearrange(
                                    "p (t k) -> p t k", k=128),
                                in1=bv_sb[:].unsqueeze(1).broadcast_to(
                                    [128, k1 - k0, 128]),
                                op=OP.add)
                            vsrc = vsum[:, 0:w]
                        else:
                            vsrc = vps[:, 0:w]
                        # pack v into bf16 at cols [t*128, t*128+64) of stg
                        dst_v = stg[:].rearrange(
                            "p (t k) -> p t k", k=128)[:, k0:k1, 0:64]
                        eng = nc.vector if (c4 % 2 == 0) else nc.scalar
                        if eng is nc.vector:
                            nc.vector.tensor_copy(
                                out=dst_v.bitcast(bf16),
                                in_=vsrc.rearrange("p (t k) -> p t k", k=128))
                        else:
                            nc.scalar.activation(
                                out=dst_v.bitcast(bf16),
                                in_=vsrc.rearrange("p (t k) -> p t k", k=128),
                                func=AF.Copy, bias=0.0, scale=1.0)
                    # pos+co cols [t*128+64, t*128+68)
                    nc.vector.tensor_copy(
                        out=stg[:].rearrange("p (t k) -> p t k", k=128)[:, 0:nt, 64:68],
                        in_=pc_sb[:].rearrange("p (t i) -> p t i", i=4)[:, t0:t1, :])
                    nc.scalar.dma_start(
                        gdst[t0 * 128:t1 * 128, :].rearrange(
                            "(t p) k -> p t k", p=128),
                        stg[:, 0:nt * 128].rearrange(
                            "p (t k) -> p t k", k=128))

            # ---------- prologue B: aw1 + pqx ----------
            for b in range(NBLK):
                pq4 = gsm.tile([128, 4], f32, tag="pq4")
                nc.vector.tensor_copy(out=pq4[:], in_=pq_sb[:, b * 4:b * 4 + 4])
                tps = gpp.tile([128, 128], f32, tag="tps", space="PSUM")
                nc.tensor.transpose(out=tps[0:4, :], in_=pq4[:], identity=idf_sb[:])
                pqT = gsm.tile([4, 128], f32, tag="pqT")
                nc.scalar.copy(out=pqT[:], in_=tps[0:4, :])
                aps = gpp2.tile([128, 128], f32, tag="aps", space="PSUM")
                nc.tensor.matmul(out=aps[:], lhsT=pqT[:], rhs=w1qc_sb[:],
                                 start=True, stop=False)
                nc.tensor.matmul(out=aps[:], lhsT=sel_sb[:], rhs=w1ocf_sb[:],
                                 start=False, stop=True)
                nc.scalar.copy(out=aw1_sb[:, b * 128:(b + 1) * 128], in_=aps[:])

            # posqext: [sh1*3, sh2*3, cqh1, cqh2] per query, bf16
            s_all = gsm.tile([128, NBLK * 3], f32, tag="s_all")
            nc.scalar.activation(
                out=s_all[:],
                in_=pq_sb[:].rearrange("p (b i) -> p b i", i=4)[:, :, 0:3],
                func=AF.Copy, bias=0.0, scale=float(inv_sig2))
            sh1 = gsm.tile([128, NBLK * 3], bf16, tag="sh1")
            nc.vector.tensor_copy(out=sh1[:], in_=s_all[:])
            sh1f = gsm.tile([128, NBLK * 3], f32, tag="sh1f")
            nc.vector.tensor_copy(out=sh1f[:], in_=sh1[:])
            sh2 = gsm.tile([128, NBLK * 3], bf16, tag="sh2")
            nc.vector.tensor_tensor(out=sh2[:], in0=s_all[:], in1=sh1f[:],
                                    op=OP.subtract)
            q2 = gsm.tile([128, NBLK * 3], f32, tag="q2")
            pqv = pq_sb[:].rearrange("p (b i) -> p b i", i=4)[:, :, 0:3]
            nc.vector.tensor_tensor(out=q2[:], in0=pqv, in1=pqv, op=OP.mult)
            cq = gsm.tile([128, NBLK], f32, tag="cq")
            nc.vector.tensor_reduce(
                out=cq[:], in_=q2[:].rearrange("p (b i) -> p b i", i=3),
                axis=mybir.AxisListType.X, op=OP.add)
            nc.scalar.activation(out=cq[:], in_=cq[:], func=AF.Copy,
                                 bias=0.0, scale=float(-inv_sig2 / 2.0))
            cqh1 = gsm.tile([128, NBLK], bf16, tag="cqh1")
            nc.vector.tensor_copy(out=cqh1[:], in_=cq[:])
            cqh1f = gsm.tile([128, NBLK], f32, tag="cqh1f")
            nc.vector.tensor_copy(out=cqh1f[:], in_=cqh1[:])
            cqh2 = gsm.tile([128, NBLK], bf16, tag="cqh2")
            nc.vector.tensor_tensor(out=cqh2[:], in0=cq[:], in1=cqh1f[:],
                                    op=OP.subtract)
            pqxv = pqx_sb[:].rearrange("p (b i) -> p b i", i=8)
            nc.vector.tensor_copy(
                out=pqxv[:, :, 0:3],
                in_=sh1[:].rearrange("p (b i) -> p b i", i=3))
            nc.vector.tensor_copy(
                out=pqxv[:, :, 3:6],
                in_=sh2[:].rearrange("p (b i) -> p b i", i=3))
            nc.vector.tensor_copy(out=pqxv[:, :, 6:7], in_=cqh1[:].unsqueeze(2))
            nc.vector.tensor_copy(out=pqxv[:, :, 7:8], in_=cqh2[:].unsqueeze(2))

        # ---------- main edge loop ----------
        mpool = ctx.enter_context(tc.tile_pool(name="main", bufs=3))
        spool = ctx.enter_context(tc.tile_pool(name="small", bufs=3))
        ppoolT = ctx.enter_context(tc.tile_pool(name="mpsT", bufs=2, space="PSUM"))
        ppoolH = ctx.enter_context(tc.tile_pool(name="mpsH", bufs=2, space="PSUM"))
        ppoolM = ctx.enter_context(tc.tile_pool(name="mpsM", bufs=2, space="PSUM"))
        opool = ctx.enter_context(tc.tile_pool(name="mpsO", bufs=2, space="PSUM"))

        for b in range(NBLK):
            pout = opool.tile([128, 132], f32, tag="pout", space="PSUM")
            for half in range(2):
                row0 = (b * 2 + half) * 128
                gsrc = GA if half == 0 else GB
                idx_sb = mpool.tile([128, NS * 8], i16, tag="idx")
                nc.sync.dma_start(idx_sb[:], idxw[row0:row0 + 128, :])
                drt = spool.tile([128, NS], f32, tag="drt")
                nc.sync.dma_start(drt[:], dstrel[row0:row0 + 128, :])
                gt = mpool.tile([128, NS * 128], f32, tag="gt")
                # HW SWDGE ring holds 1024 descriptors; chunk at 1024.
                gv = gt[:].rearrange("p (n k) -> p n k", k=128)
                for c0 in range(0, NS, 8):
                    c1 = min(c0 + 8, NS)
                    n_c = (c1 - c0) * 128
                    nc.gpsimd.dma_gather(
                        out_ap=gv[:, c0:c1, :],
                        in_ap=gsrc[:],
                        idxs_ap=idx_sb[:, c0 * 8:c0 * 8 + n_c // 16],
                        num_idxs=n_c,
                        num_idxs_reg=n_c,
                        elem_size=128,
                    )
                gtb = gt[:].bitcast(bf16)  # [128, NS*256]

                # one-hot mask [edge, query] + obs cols 124:128
                mext = mpool.tile([128, NS * 128], bf16, tag="mext")
                for n in range(NS):
                    nc.vector.tensor_scalar(
                        out=mext[:, n * 128:n * 128 + 124],
                        in0=iota_sb[:, 0:124],
                        scalar1=drt[:, n:n + 1], scalar2=None,
                        op0=OP.is_equal)
                nc.vector.tensor_copy(
                    out=mext[:].rearrange("p (n k) -> p n k", k=128)[:, :, 124:128],
                    in_=gv[:, :, 64:68])

                # transposed stack via PE transpose + batched PSUM->SBUF copies
                stack = mpool.tile([128, NS * 128], bf16, tag="stack")
                for ch in range(NCH):
                    n0 = ch * 4
                    n1 = min(n0 + 4, NS)
                    tps = ppoolT.tile([128, 512], bf16, tag="tps", space="PSUM")
                    for n in range(n0, n1):
                        nc.tensor.transpose(
                            out=tps[:, (n - n0) * 128:(n - n0 + 1) * 128],
                            in_=mext[:, n * 128:(n + 1) * 128],
                            identity=idb_sb[:])
                    w = (n1 - n0) * 128
                    if ch % 2 == 0:
                        nc.scalar.copy(out=stack[:, n0 * 128:n0 * 128 + w],
                                       in_=tps[:, 0:w])
                    else:
                        nc.vector.tensor_copy(out=stack[:, n0 * 128:n0 * 128 + w],
                                              in_=tps[:, 0:w])

                hid = mpool.tile([128, NS * 128], bf16, tag="hid")
                pmisc = ppoolM.tile([128, NS * 12], f32, tag="pmisc", space="PSUM")
                qdv = pmisc[:].rearrange("p (n k) -> p n k", k=12)
                for ch in range(NCH):
                    n0 = ch * 4
                    n1 = min(n0 + 4, NS)
                    phid = ppoolH.tile([128, 512], f32, tag="phid", space="PSUM")
                    for n in range(n0, n1):
                        nc.tensor.matmul(
                            out=phid[:, (n - n0) * 128:(n - n0 + 1) * 128],
                            lhsT=aw1_sb[:, b * 128:(b + 1) * 128],
                            rhs=stack[:, n * 128:(n + 1) * 128],
                            start=True, stop=True)
                    w = (n1 - n0) * 128
                    nc.scalar.activation(
                        out=hid[:, n0 * 128:n0 * 128 + w], in_=phid[:, 0:w],
                        func=AF.Relu, bias=0.0, scale=1.0)
                for n in range(NS):
                    nc.tensor.matmul(
                        out=qdv[:, n, 0:8],
                        lhsT=stack[:, n * 128:(n + 1) * 128],
                        rhs=pqx_sb[:, b * 8:b * 8 + 8],
                        start=True, stop=True)
                for n in range(NS):
                    nc.tensor.matmul(
                        out=qdv[:, n, 8:12],
                        lhsT=hid[:, n * 128:(n + 1) * 128],
                        rhs=w2_sb[:],
                        start=True, stop=True)

                # d = sum_i (sh1_i+sh2_i)*o_i + cqh1 + cqh2 + co  (co has +SHIFT)
                ov = gv[:, :, 64:67]
                ta = spool.tile([128, NS * 3], f32, tag="ta")
                nc.vector.tensor_tensor(
                    out=ta[:].rearrange("p (n i) -> p n i", i=3),
                    in0=qdv[:, :, 0:3], in1=ov, op=OP.mult)
                tb = spool.tile([128, NS * 3], f32, tag="tb")
                nc.vector.tensor_tensor(
                    out=tb[:].rearrange("p (n i) -> p n i", i=3),
                    in0=qdv[:, :, 3:6], in1=ov, op=OP.mult)
                tab = spool.tile([128, NS * 3], f32, tag="tab")
                nc.vector.tensor_tensor(out=tab[:], in0=ta[:], in1=tb[:],
                                        op=OP.add)
                dsum = spool.tile([128, NS], f32, tag="dsum")
                nc.vector.tensor_reduce(
                    out=dsum[:], in_=tab[:].rearrange("p (n i) -> p n i", i=3),
                    axis=mybir.AxisListType.X, op=OP.add)
                e2 = spool.tile([128, NS], f32, tag="e2")
                nc.vector.tensor_tensor(out=e2[:], in0=dsum[:],
                                        in1=qdv[:, :, 6:7].squeeze(2), op=OP.add)
                e3 = spool.tile([128, NS], f32, tag="e3")
                nc.vector.tensor_tensor(out=e3[:], in0=e2[:],
                                        in1=qdv[:, :, 7:8].squeeze(2), op=OP.add)
                e4 = spool.tile([128, NS], f32, tag="e4")
                nc.vector.tensor_tensor(out=e4[:], in0=e3[:],
                                        in1=gv[:, :, 67:68].squeeze(2), op=OP.add)
                lst = spool.tile([128, NS * 4], f32, tag="lst")
                nc.vector.tensor_tensor(
                    out=lst[:].rearrange("p (n h) -> p n h", h=4),
                    in0=qdv[:, :, 8:12],
                    in1=e4[:].unsqueeze(2).broadcast_to([128, NS, 4]),
                    op=OP.add)
                if has_b1b2v:
                    nc.vector.tensor_tensor(
                        out=lst[:].rearrange("p (n h) -> p n h", h=4),
                        in0=lst[:].rearrange("p (n h) -> p n h", h=4),
                        in1=b2_sb[:].unsqueeze(1).broadcast_to([128, NS, 4]),
                        op=OP.add)
                exw = spool.tile([128, NS * 4], bf16, tag="exw")
                nc.scalar.activation(out=exw[:], in_=lst[:], func=AF.Exp,
                                     bias=0.0, scale=1.0)

                # vse: head-interleaved v * exp, plus denom cols
                vse = mpool.tile([128, NS * 132], bf16, tag="vse")
                vsev = vse[:].rearrange("p (n k) -> p n k", k=132)
                nc.vector.tensor_tensor(
                    out=vsev[:, :, 0:128].rearrange("p n (w h) -> p n w h", h=4),
                    in0=gtb.rearrange("p (n k) -> p n k", k=256)[:, :, 0:128]
                        .rearrange("p n (w h) -> p n w h", h=4),
                    in1=exw[:].rearrange("p (n h) -> p n h", h=4).unsqueeze(2)
                        .broadcast_to([128, NS, 32, 4]),
                    op=OP.mult)
                nc.vector.tensor_copy(
                    out=vsev[:, :, 128:132],
                    in_=exw[:].rearrange("p (n h) -> p n h", h=4))
                for n in range(NS):
                    nc.tensor.matmul(
                        out=pout[:],
                        lhsT=mext[:, n * 128:(n + 1) * 128],
                        rhs=vse[:, n * 132:(n + 1) * 132],
                        start=(half == 0 and n == 0),
                        stop=(half == 1 and n == NS - 1))

            den = spool.tile([128, 4], f32, tag="den")
            nc.scalar.activation(out=den[:], in_=pout[:, 128:132],
                                 func=AF.Copy, bias=1e-30, scale=1.0)
            rec = spool.tile([128, 4], f32, tag="rec")
            nc.vector.reciprocal(out=rec[:], in_=den[:])
            onorm = spool.tile([128, 128], f32, tag="onorm")
            nc.vector.tensor_tensor(
                out=onorm[:].rearrange("p (w h) -> p w h", h=4),
                in0=pout[:, 0:128].rearrange("p (w h) -> p w h", h=4),
                in1=rec[:].unsqueeze(1).broadcast_to([128, 32, 4]),
                op=OP.mult)
            nc.sync.dma_start(out_d[b * QB:(b + 1) * QB, :], onorm[0:QB, :])

    nc.compile()
    return nc


def _host_prep(h_obs, pos_obs, pos_query, src, dst, W1, b1, W2, b2, Wv, bv,
               log_sigma):
    import ml_dtypes
    bf = ml_dtypes.bfloat16

    src = np.asarray(src).astype(np.int64)
    dst = np.asarray(dst).astype(np.int64)
    h_obs = np.asarray(h_obs, dtype=np.float32)
    pos_obs = np.asarray(pos_obs, dtype=np.float32)
    pos_query = np.asarray(pos_query, dtype=np.float32)
    W1 = np.asarray(W1, dtype=np.float32)
    W2 = np.asarray(W2, dtype=np.float32)
    Wv = np.asarray(Wv, dtype=np.float32)
    b1 = np.asarray(b1, dtype=np.float32)
    b2 = np.asarray(b2, dtype=np.float32)
    bv = np.asarray(bv, dtype=np.float32)
    sigma = np.exp(np.float32(log_sigma)) + np.float32(1e-6)
    inv_sig2 = float(1.0 / (np.float64(sigma) ** 2))

    # per-core edge partition, then per (core, block, half) lists
    core_lists = []
    max_ns = 1
    edge_bounds = np.searchsorted(dst, np.arange(NCORES + 1) * QPC)
    for c in range(NCORES):
        e0, e1 = edge_bounds[c], edge_bounds[c + 1]
        dl = dst[e0:e1] - c * QPC
        sl = src[e0:e1]
        blocks = []
        blk_bounds = np.searchsorted(dl, np.arange(NBLK + 1) * QB)
        for b in range(NBLK):
            be0, be1 = blk_bounds[b], blk_bounds[b + 1]
            bsrc = sl[be0:be1]
            bdr = dl[be0:be1] - b * QB
            m = bsrc < HALF
            halves = []
            for hm, off in ((m, 0), (~m, HALF)):
                s_h = (bsrc[hm] - off).astype(np.int16)
                d_h = bdr[hm].astype(np.float32)
                halves.append((s_h, d_h))
                max_ns = max(max_ns, math.ceil(max(len(s_h), 1) / 128))
            blocks.append(halves)
        core_lists.append(blocks)
    NS = max_ns
    NSP = NS * 128

    iota = np.broadcast_to(np.arange(128, dtype=np.float32), (128, 128))
    ident = np.eye(128, dtype=np.float32)
    w1qcb1 = np.concatenate([W1[0:3] + W1[3:6], b1[None, :]], 0).astype(np.float32)
    w1oc = np.zeros((4, 128), np.float32)
    w1oc[0:3] = W1[6:9] - W1[0:3]
    has_b1b2v = bool(np.any(b1) or np.any(b2) or np.any(bv))

    # head-interleaved Wv / bv: col w*4+h <- h*32+w
    wv_int = Wv.reshape(128, HEADS, HEAD_DIM).transpose(0, 2, 1).reshape(128, 128)
    bv_int = bv.reshape(HEADS, HEAD_DIM).T.reshape(128)

    # host-transposed h (bf16) per half: [128, HROWS]
    hTA = np.zeros((128, HROWS), bf)
    hTA[:, :HALF] = h_obs[:HALF].T.astype(bf)
    hTB = np.zeros((128, HROWS), bf)
    hTB[:, :N_O - HALF] = h_obs[HALF:].T.astype(bf)
    # pos packed [128, NTIL*4]: col 4t+i = pos[t*128+p, i]; col 4t+3 = 0 (co slot)
    def pos_pack(p):
        pp = np.zeros((HROWS, 4), np.float32)
        pp[:len(p), 0:3] = p
        return pp.reshape(NTIL, 128, 4).transpose(1, 0, 2).reshape(128, NTIL * 4)
    pcA = pos_pack(pos_obs[:HALF])
    pcB = pos_pack(pos_obs[HALF:])

    in_maps = []
    for c in range(NCORES):
        idxw = np.zeros((NBLK * 2, 128, NS * 8), np.int16)
        drel = np.full((NBLK * 2, 128, NS), -1.0, np.float32)
        for b in range(NBLK):
            for half in range(2):
                s_h, d_h = core_lists[c][b][half]
                n = len(s_h)
                ip = np.zeros(NSP, np.int16)
                ip[:n] = s_h
                w = ip.reshape(NSP // 16, 16).T  # [16, NS*8]
                idxw[b * 2 + half] = np.tile(w, (8, 1))
                dp = np.full(NSP, -1.0, np.float32)
                dp[:n] = d_h
                drel[b * 2 + half] = dp.reshape(NS, 128).T
        pqb = np.zeros((128, NBLK * 4), np.float32)
        qs = pos_query[c * QPC:(c + 1) * QPC]
        for b in range(NBLK):
            lo, hi = b * QB, min((b + 1) * QB, QPC)
            pqb[:hi - lo, b * 4:b * 4 + 3] = qs[lo:hi]
            pqb[:hi - lo, b * 4 + 3] = 1.0
        in_maps.append({
            "hTA": hTA, "hTB": hTB, "pcA": pcA, "pcB": pcB,
            "posq_blk": pqb,
            "idxw": idxw.reshape(NBLK * 2 * 128, NS * 8),
            "dstrel": drel.reshape(NBLK * 2 * 128, NS),
            "w1qcb1": w1qcb1,
            "w1ocf": w1oc,
            "sel124": np.eye(128, dtype=np.float32)[124:128],
            "w2": W2.astype(bf),
            "wv": wv_int.astype(bf),
            "b2rep": np.broadcast_to(b2, (128, 4)).copy().astype(np.float32),
            "bvrep": np.broadcast_to(bv_int, (128, 128)).copy().astype(np.float32),
            "iota_in": iota.astype(bf),
            "idf32": ident,
            "idbf": ident.astype(bf),
        })
    return NS, inv_sig2, has_b1b2v, in_maps


def kernel(h_obs, pos_obs, pos_query, src, dst, W1, b1, W2, b2, Wv, bv,
           log_sigma, **_unused):
    import sys
    for p in ("/opt/trn_rl_repo", "/root/.axon_site/_ro/trn_rl_repo"):
        if p not in sys.path:
            sys.path.append(p)
    from concourse.bass_utils import run_bass_kernel_spmd

    NS, inv_sig2, has_b1b2v, in_maps = _host_prep(
        h_obs, pos_obs, pos_query, src, dst, W1, b1, W2, b2, Wv, bv, log_sigma)

    key = (NS, round(inv_sig2, 9), has_b1b2v)
    if key not in _PROG_CACHE:
        _PROG_CACHE[key] = _build_program(NS, inv_sig2, has_b1b2v)
    nc = _PROG_CACHE[key]

    res = run_bass_kernel_spmd(nc, in_maps, core_ids=list(range(NCORES)))
    outs = [np.asarray(r["out"])[:QPC] for r in res.results]
    full = np.concatenate(outs, axis=0).astype(np.float32)
    # un-interleave heads: col w*4+h -> h*32+w
    return np.ascontiguousarray(
        full.reshape(-1, HEAD_DIM, HEADS).transpose(0, 2, 1).reshape(-1, 128))


if __name__ == "__main__":
    pass


# revision 7
# speedup vs baseline: 3.0009x; 1.0876x over previous
"""Bipartite GNN message-passing kernel for Trainium2 (8 NeuronCores).

Strategy (v3):
  - dst is sorted -> shard queries (50000/8=6250 per core); each core gets a
    contiguous edge range. No cross-core reduction needed.
  - Per core, queries are processed in blocks of 124. Per block, edges are
    split into two halves by src (<25000 / >=25000) so gather indices fit in
    int16, padded to a uniform NS*128 subtile structure.
  - A packed per-obs table (v in bf16 head-interleaved, pos_obs f32,
    co = -inv2/2*|o|^2 + C_SHIFT; 512B rows) is built on device from a
    host-pretransposed bf16 h, then rows are fetched per edge with
    dma_gather.
  - Per-edge query one-hot (mext) is built with per-subtile tensor_scalar
    is_equal (4x DVE mode); transposed stack via PE transpose + batched
    PSUM->SBUF copies split across Act/DVE/Pool.
  - v is stored head-interleaved (col = w*4+h) so the attention scaling
    multiply has a packed 4-wide inner dim (2x DVE mode). Host un-interleaves
    the output.
  - Softmax runs without max-subtraction: logits <= ~2 and a +60 shift
    (folded into co) keeps every denominator >> 1e-16 while exp stays in
    fp32/bf16 range.
"""

import math
import numpy as np

N_O = 50000
N_Q = 50000
E_TOT = 1_600_000
LATENT = 128
HEADS = 4
HEAD_DIM = 32
NCORES = 8
QPC = N_Q // NCORES          # queries per core
QB = 124                     # queries per block (124 + 4 obs-feature rows = 128)
NBLK = math.ceil(QPC / QB)   # 51
HALF = 25000                 # src split point for int16 indices
HROWS = 25088                # 196*128, padded rows per half-table
NTIL = HROWS // 128          # 196 tiles per half
C_SHIFT = 60.0
TB = 16                      # table-build tiles per DMA batch

_PROG_CACHE = {}


def _build_program(NSL, inv_sig2, has_b1b2v):
    import concourse.bacc as bacc
    import concourse.bass as bass
    import concourse.mybir as mybir
    import concourse.tile as tile
    from contextlib import ExitStack

    dt = mybir.dt
    f32, bf16, i16 = dt.float32, dt.bfloat16, dt.int16
    AF = mybir.ActivationFunctionType
    OP = mybir.AluOpType
    NS = max(max(r) for r in NSL)  # max subtiles (tile sizing)

    nc = bacc.Bacc("TRN2", target_bir_lowering=False, debug=False)

    # ---- DRAM tensors (per-core inputs) ----
    hTA = nc.dram_tensor("hTA", [128, HROWS], bf16, kind="ExternalInput")
    hTB = nc.dram_tensor("hTB", [128, HROWS], bf16, kind="ExternalInput")
    pcA = nc.dram_tensor("pcA", [128, NTIL * 4], f32, kind="ExternalInput")
    pcB = nc.dram_tensor("pcB", [128, NTIL * 4], f32, kind="ExternalInput")
    posq_blk = nc.dram_tensor("posq_blk", [128, NBLK * 4], f32, kind="ExternalInput")
    idxw = nc.dram_tensor("idxw", [NBLK * 2 * 128, NS * 8], i16, kind="ExternalInput")
    dstrel = nc.dram_tensor("dstrel", [NBLK * 2 * 128, NS], f32, kind="ExternalInput")
    w1qcb1 = nc.dram_tensor("w1qcb1", [4, 128], f32, kind="ExternalInput")
    w1ocf = nc.dram_tensor("w1ocf", [4, 128], f32, kind="ExternalInput")
    sel124 = nc.dram_tensor("sel124", [4, 128], f32, kind="ExternalInput")
    w2 = nc.dram_tensor("w2", [128, 4], bf16, kind="ExternalInput")
    wv = nc.dram_tensor("wv", [128, 128], bf16, kind="ExternalInput")  # interleaved
    b2rep = nc.dram_tensor("b2rep", [128, 4], f32, kind="ExternalInput")
    bvrep = nc.dram_tensor("bvrep", [128, 128], f32, kind="ExternalInput")  # interleaved
    iota_in = nc.dram_tensor("iota_in", [128, 128], bf16, kind="ExternalInput")
    idf32 = nc.dram_tensor("idf32", [128, 128], f32, kind="ExternalInput")
    idbf = nc.dram_tensor("idbf", [128, 128], bf16, kind="ExternalInput")

    GA = nc.dram_tensor("GA", [HROWS, 128], f32)   # packed table half A
    GB = nc.dram_tensor("GB", [HROWS, 128], f32)   # packed table half B
    out_d = nc.dram_tensor("out", [NBLK * QB, 128], f32, kind="ExternalOutput")

    with tile.TileContext(nc) as tc, ExitStack() as ctx:
        cpool = ctx.enter_context(tc.tile_pool(name="consts", bufs=1))
        aw1_sb = cpool.tile([128, NBLK * 128], bf16, tag="aw1")
        pqx_sb = cpool.tile([128, NBLK * 8], bf16, tag="pqx")
        pqc_sb = cpool.tile([128, NBLK * 8], bf16, tag="pqc")

        iota_sb = cpool.tile([128, 128], bf16, tag="iota")
        nc.sync.dma_start(iota_sb[:], iota_in[:])
        idf_sb = cpool.tile([128, 128], f32, tag="idf")
        nc.sync.dma_start(idf_sb[:], idf32[:])
        idb_sb = cpool.tile([128, 128], bf16, tag="idb")
        nc.sync.dma_start(idb_sb[:], idbf[:])
        w1qc_sb = cpool.tile([4, 128], f32, tag="w1qc")
        nc.sync.dma_start(w1qc_sb[:], w1qcb1[:])
        w1ocf_sb = cpool.tile([4, 128], f32, tag="w1ocf")
        nc.sync.dma_start(w1ocf_sb[:], w1ocf[:])
        sel_sb = cpool.tile([4, 128], f32, tag="sel")
        nc.sync.dma_start(sel_sb[:], sel124[:])
        w2_sb = cpool.tile([128, 4], bf16, tag="w2")
        nc.sync.dma_start(w2_sb[:], w2[:])
        wv_sb = cpool.tile([128, 128], bf16, tag="wv")
        nc.sync.dma_start(wv_sb[:], wv[:])
        b2_sb = cpool.tile([128, 4], f32, tag="b2")
        nc.sync.dma_start(b2_sb[:], b2rep[:])
        bv_sb = cpool.tile([128, 128], f32, tag="bv")
        nc.sync.dma_start(bv_sb[:], bvrep[:])
        pq_sb = cpool.tile([128, NBLK * 4], f32, tag="pq")
        nc.sync.dma_start(pq_sb[:], posq_blk[:])
        pcA_sb = cpool.tile([128, NTIL * 4], f32, tag="pcA")
        nc.sync.dma_start(pcA_sb[:], pcA[:])
        pcB_sb = cpool.tile([128, NTIL * 4], f32, tag="pcB")
        nc.sync.dma_start(pcB_sb[:], pcB[:])

        # ---------- prologue A: packed obs table (v | pos | co) ----------
        nbat = math.ceil(NTIL / TB)
        with tc.tile_pool(name="gb_sb", bufs=2) as gp, \
             tc.tile_pool(name="gb_sm", bufs=2) as gsm, \
             tc.tile_pool(name="gb_ps", bufs=2, space="PSUM") as gpp, \
             tc.tile_pool(name="gb_ps2", bufs=2, space="PSUM") as gpp2:
            for t_half, (hsrc, pc_sb, gdst) in enumerate(
                    ((hTA, pcA_sb, GA), (hTB, pcB_sb, GB))):
                # co into col 4t+3 of the resident pos tile:
                # co = -inv2/2 * (o0^2+o1^2+o2^2) + C_SHIFT
                pcv = pc_sb[:].rearrange("p (t i) -> p t i", i=4)
                sq = gsm.tile([128, NTIL * 3], f32, tag="sq")
                nc.vector.tensor_tensor(
                    out=sq[:].rearrange("p (t i) -> p t i", i=3),
                    in0=pcv[:, :, 0:3], in1=pcv[:, :, 0:3], op=OP.mult)
                co = gsm.tile([128, NTIL], f32, tag="co")
                nc.vector.tensor_reduce(
                    out=co[:], in_=sq[:].rearrange("p (t i) -> p t i", i=3),
                    axis=mybir.AxisListType.X, op=OP.add)
                nc.scalar.activation(
                    out=pcv[:, :, 3:4].squeeze(2), in_=co[:], func=AF.Copy,
                    bias=float(C_SHIFT), scale=float(-inv_sig2 / 2.0))

                for bt in range(nbat):
                    t0 = bt * TB
                    t1 = min(t0 + TB, NTIL)
                    nt = t1 - t0
                    hb = gp.tile([128, TB * 128], bf16, tag="hb")
                    nc.sync.dma_start(hb[:, 0:nt * 128],
                                      hsrc[:, t0 * 128:t1 * 128])
                    stg = gp.tile([128, TB * 128], f32, tag="stg")
                    for c4 in range(math.ceil(nt / 4)):
                        k0 = c4 * 4
                        k1 = min(k0 + 4, nt)
                        vps = gpp.tile([128, 512], f32, tag="vps", space="PSUM")
                        for k in range(k0, k1):
                            nc.tensor.matmul(
                                out=vps[:, (k - k0) * 128:(k - k0 + 1) * 128],
                                lhsT=hb[:, k * 128:(k + 1) * 128],
                                rhs=wv_sb[:], start=True, stop=True)
                        w = (k1 - k0) * 128
                        if has_b1b2v:
                            vsum = gsm.tile([128, 512], f32, tag="vsum")
                            nc.vector.tensor_tensor(
                                out=vsum[0:128, 0:w].rearrange(
                                    "p (t k) -> p t k", k=128),
                                in0=vps[:, 0:w].rearrange(
                                    "p (t k) -> p t k", k=128),
                                in1=bv_sb[:].unsqueeze(1).broadcast_to(
                                    [128, k1 - k0, 128]),
                                op=OP.add)
                            vsrc = vsum[:, 0:w]
                        else:
                            vsrc = vps[:, 0:w]
                        # pack v into bf16 at cols [t*128, t*128+64) of stg
                        dst_v = stg[:].rearrange(
                            "p (t k) -> p t k", k=128)[:, k0:k1, 0:64]
                        eng = nc.vector if (c4 % 2 == 0) else nc.scalar
                        if eng is nc.vector:
                            nc.vector.tensor_copy(
                                out=dst_v.bitcast(bf16),
                                in_=vsrc.rearrange("p (t k) -> p t k", k=128))
                        else:
                            nc.scalar.activation(
                                out=dst_v.bitcast(bf16),
                                in_=vsrc.rearrange("p (t k) -> p t k", k=128),
                                func=AF.Copy, bias=0.0, scale=1.0)
                    # pos+co cols [t*128+64, t*128+68)
                    nc.vector.tensor_copy(
                        out=stg[:].rearrange("p (t k) -> p t k", k=128)[:, 0:nt, 64:68],
                        in_=pc_sb[:].rearrange("p (t i) -> p t i", i=4)[:, t0:t1, :])
                    nc.scalar.dma_start(
                        gdst[t0 * 128:t1 * 128, :].rearrange(
                            "(t p) k -> p t k", p=128),
                        stg[:, 0:nt * 128].rearrange(
                            "p (t k) -> p t k", k=128))

            # ---------- prologue B: aw1 + pqx ----------
            for b in range(NBLK):
                pq4 = gsm.tile([128, 4], f32, tag="pq4")
                nc.vector.tensor_copy(out=pq4[:], in_=pq_sb[:, b * 4:b * 4 + 4])
                tps = gpp.tile([128, 128], f32, tag="tps", space="PSUM")
                nc.tensor.transpose(out=tps[0:4, :], in_=pq4[:], identity=idf_sb[:])
                pqT = gsm.tile([4, 128], f32, tag="pqT")
                nc.scalar.copy(out=pqT[:], in_=tps[0:4, :])
                aps = gpp2.tile([128, 128], f32, tag="aps", space="PSUM")
                nc.tensor.matmul(out=aps[:], lhsT=pqT[:], rhs=w1qc_sb[:],
                                 start=True, stop=False)
                nc.tensor.matmul(out=aps[:], lhsT=sel_sb[:], rhs=w1ocf_sb[:],
                                 start=False, stop=True)
                nc.scalar.copy(out=aw1_sb[:, b * 128:(b + 1) * 128], in_=aps[:])

            # posqext: [sh1*3, sh2*3, cqh1, cqh2] per query, bf16
            s_all = gsm.tile([128, NBLK * 3], f32, tag="s_all")
            nc.scalar.activation(
                out=s_all[:],
                in_=pq_sb[:].rearrange("p (b i) -> p b i", i=4)[:, :, 0:3],
                func=AF.Copy, bias=0.0, scale=float(inv_sig2))
            sh1 = gsm.tile([128, NBLK * 3], bf16, tag="sh1")
            nc.vector.tensor_copy(out=sh1[:], in_=s_all[:])
            sh1f = gsm.tile([128, NBLK * 3], f32, tag="sh1f")
            nc.vector.tensor_copy(out=sh1f[:], in_=sh1[:])
            sh2 = gsm.tile([128, NBLK * 3], bf16, tag="sh2")
            nc.vector.tensor_tensor(out=sh2[:], in0=s_all[:], in1=sh1f[:],
                                    op=OP.subtract)
            q2 = gsm.tile([128, NBLK * 3], f32, tag="q2")
            pqv = pq_sb[:].rearrange("p (b i) -> p b i", i=4)[:, :, 0:3]
            nc.vector.tensor_tensor(out=q2[:], in0=pqv, in1=pqv, op=OP.mult)
            cq = gsm.tile([128, NBLK], f32, tag="cq")
            nc.vector.tensor_reduce(
                out=cq[:], in_=q2[:].rearrange("p (b i) -> p b i", i=3),
                axis=mybir.AxisListType.X, op=OP.add)
            nc.scalar.activation(out=cq[:], in_=cq[:], func=AF.Copy,
                                 bias=0.0, scale=float(-inv_sig2 / 2.0))
            cqh1 = gsm.tile([128, NBLK], bf16, tag="cqh1")
            nc.vector.tensor_copy(out=cqh1[:], in_=cq[:])
            cqh1f = gsm.tile([128, NBLK], f32, tag="cqh1f")
            nc.vector.tensor_copy(out=cqh1f[:], in_=cqh1[:])
            cqh2 = gsm.tile([128, NBLK], bf16, tag="cqh2")
            nc.vector.tensor_tensor(out=cqh2[:], in0=cq[:], in1=cqh1f[:],
                                    op=OP.subtract)
            pqxv = pqx_sb[:].rearrange("p (b i) -> p b i", i=8)
            nc.vector.tensor_copy(
                out=pqxv[:, :, 0:3],
                in_=sh1[:].rearrange("p (b i) -> p b i", i=3))
            nc.vector.tensor_copy(
                out=pqxv[:, :, 3:6],
                in_=sh2[:].rearrange("p (b i) -> p b i", i=3))
            nc.vector.tensor_copy(out=pqxv[:, :, 6:7], in_=cqh1[:].unsqueeze(2))
            nc.vector.tensor_copy(out=pqxv[:, :, 7:8], in_=cqh2[:].unsqueeze(2))
            pqcv = pqc_sb[:].rearrange("p (b i) -> p b i", i=8)
            nc.vector.tensor_copy(
                out=pqcv[:, :, 0:4],
                in_=cqh1[:].unsqueeze(2).broadcast_to([128, NBLK, 4]))
            nc.vector.tensor_copy(
                out=pqcv[:, :, 4:8],
                in_=cqh2[:].unsqueeze(2).broadcast_to([128, NBLK, 4]))

        # ---------- main edge loop ----------
        mpool = ctx.enter_context(tc.tile_pool(name="main", bufs=3))
        spool = ctx.enter_context(tc.tile_pool(name="small", bufs=3))
        ppoolT = ctx.enter_context(tc.tile_pool(name="mpsT", bufs=2, space="PSUM"))
        ppoolH = ctx.enter_context(tc.tile_pool(name="mpsH", bufs=2, space="PSUM"))
        ppoolM = ctx.enter_context(tc.tile_pool(name="mpsM", bufs=2, space="PSUM"))
        opool = ctx.enter_context(tc.tile_pool(name="mpsO", bufs=2, space="PSUM"))

        for b in range(NBLK):
            pout = opool.tile([128, 132], f32, tag="pout", space="PSUM")
            for half in range(2):
                NSB = NSL[b][half]
                NCH = math.ceil(NSB / 4)
                row0 = (b * 2 + half) * 128
                gsrc = GA if half == 0 else GB
                idx_sb = mpool.tile([128, NS * 8], i16, tag="idx")
                nc.sync.dma_start(idx_sb[:, 0:NSB * 8],
                                  idxw[row0:row0 + 128, 0:NSB * 8])
                drt = spool.tile([128, NS], f32, tag="drt")
                nc.sync.dma_start(drt[:, 0:NSB], dstrel[row0:row0 + 128, 0:NSB])
                gt = mpool.tile([128, NS * 128], f32, tag="gt")
                # HW SWDGE ring holds 1024 descriptors; chunk at 1024.
                gv = gt[:].rearrange("p (n k) -> p n k", k=128)
                for c0 in range(0, NSB, 8):
                    c1 = min(c0 + 8, NSB)
                    n_c = (c1 - c0) * 128
                    nc.gpsimd.dma_gather(
                        out_ap=gv[:, c0:c1, :],
                        in_ap=gsrc[:],
                        idxs_ap=idx_sb[:, c0 * 8:c0 * 8 + n_c // 16],
                        num_idxs=n_c,
                        num_idxs_reg=n_c,
                        elem_size=128,
                    )
                gtb = gt[:].bitcast(bf16)  # [128, NS*256]

                # one-hot mask [edge, query] + obs cols 124:128
                mext = mpool.tile([128, NS * 128], bf16, tag="mext")
                for n in range(NSB):
                    nc.vector.tensor_scalar(
                        out=mext[:, n * 128:n * 128 + 124],
                        in0=iota_sb[:, 0:124],
                        scalar1=drt[:, n:n + 1], scalar2=None,
                        op0=OP.is_equal)
                nc.vector.tensor_copy(
                    out=mext[:].rearrange("p (n k) -> p n k", k=128)[:, 0:NSB, 124:128],
                    in_=gv[:, 0:NSB, 64:68])

                # transposed stack via PE transpose + batched PSUM->SBUF copies
                stack = mpool.tile([128, NS * 128], bf16, tag="stack")
                for ch in range(NCH):
                    n0 = ch * 4
                    n1 = min(n0 + 4, NSB)
                    tps = ppoolT.tile([128, 512], bf16, tag="tps", space="PSUM")
                    for n in range(n0, n1):
                        nc.tensor.transpose(
                            out=tps[:, (n - n0) * 128:(n - n0 + 1) * 128],
                            in_=mext[:, n * 128:(n + 1) * 128],
                            identity=idb_sb[:])
                    w = (n1 - n0) * 128
                    if ch % 2 == 0:
                        nc.scalar.copy(out=stack[:, n0 * 128:n0 * 128 + w],
                                       in_=tps[:, 0:w])
                    else:
                        nc.vector.tensor_copy(out=stack[:, n0 * 128:n0 * 128 + w],
                                              in_=tps[:, 0:w])

                hid = mpool.tile([128, NS * 128], bf16, tag="hid")
                pmisc = ppoolM.tile([128, NS * 8], f32, tag="pmisc", space="PSUM")
                qdv = pmisc[:].rearrange("p (n k) -> p n k", k=8)
                for ch in range(NCH):
                    n0 = ch * 4
                    n1 = min(n0 + 4, NSB)
                    phid = ppoolH.tile([128, 512], f32, tag="phid", space="PSUM")
                    for n in range(n0, n1):
                        nc.tensor.matmul(
                            out=phid[:, (n - n0) * 128:(n - n0 + 1) * 128],
                            lhsT=aw1_sb[:, b * 128:(b + 1) * 128],
                            rhs=stack[:, n * 128:(n + 1) * 128],
                            start=True, stop=True)
                    w = (n1 - n0) * 128
                    nc.scalar.activation(
                        out=hid[:, n0 * 128:n0 * 128 + w], in_=phid[:, 0:w],
                        func=AF.Relu, bias=0.0, scale=1.0)
                for n in range(NSB):
                    # s = sh1 + sh2, accumulated in psum
                    nc.tensor.matmul(
                        out=qdv[:, n, 0:3],
                        lhsT=stack[:, n * 128:(n + 1) * 128],
                        rhs=pqx_sb[:, b * 8:b * 8 + 3],
                        start=True, stop=False)
                    nc.tensor.matmul(
                        out=qdv[:, n, 0:3],
                        lhsT=stack[:, n * 128:(n + 1) * 128],
                        rhs=pqx_sb[:, b * 8 + 3:b * 8 + 6],
                        start=False, stop=True)
                    # logits cols 4:8: cqh1 + cqh2 + w2.hid
                    nc.tensor.matmul(
                        out=qdv[:, n, 4:8],
                        lhsT=stack[:, n * 128:(n + 1) * 128],
                        rhs=pqc_sb[:, b * 8:b * 8 + 4],
                        start=True, stop=False)
                    nc.tensor.matmul(
                        out=qdv[:, n, 4:8],
                        lhsT=stack[:, n * 128:(n + 1) * 128],
                        rhs=pqc_sb[:, b * 8 + 4:b * 8 + 8],
                        start=False, stop=False)
                    nc.tensor.matmul(
                        out=qdv[:, n, 4:8],
                        lhsT=hid[:, n * 128:(n + 1) * 128],
                        rhs=w2_sb[:],
                        start=False, stop=True)

                # d = sum_i s_i*o_i + co   (cq folded into logits psum)
                ov = gv[:, 0:NSB, 64:67]
                ta = spool.tile([128, NS * 3], f32, tag="ta")
                nc.vector.tensor_tensor(
                    out=ta[:, 0:NSB * 3].rearrange("p (n i) -> p n i", i=3),
                    in0=qdv[:, 0:NSB, 0:3], in1=ov, op=OP.mult)
                dsum = spool.tile([128, NS], f32, tag="dsum")
                nc.vector.tensor_reduce(
                    out=dsum[:, 0:NSB],
                    in_=ta[:, 0:NSB * 3].rearrange("p (n i) -> p n i", i=3),
                    axis=mybir.AxisListType.X, op=OP.add)
                e4 = spool.tile([128, NS], f32, tag="e4")
                nc.vector.tensor_tensor(out=e4[:, 0:NSB], in0=dsum[:, 0:NSB],
                                        in1=gv[:, 0:NSB, 67:68].squeeze(2),
                                        op=OP.add)
                lst = spool.tile([128, NS * 4], f32, tag="lst")
                nc.vector.tensor_tensor(
                    out=lst[:, 0:NSB * 4].rearrange("p (n h) -> p n h", h=4),
                    in0=qdv[:, 0:NSB, 4:8],
                    in1=e4[:, 0:NSB].unsqueeze(2).broadcast_to([128, NSB, 4]),
                    op=OP.add)
                if has_b1b2v:
                    nc.vector.tensor_tensor(
                        out=lst[:, 0:NSB * 4].rearrange("p (n h) -> p n h", h=4),
                        in0=lst[:, 0:NSB * 4].rearrange("p (n h) -> p n h", h=4),
                        in1=b2_sb[:].unsqueeze(1).broadcast_to([128, NSB, 4]),
                        op=OP.add)
                exw = spool.tile([128, NS * 4], bf16, tag="exw")
                nc.scalar.activation(out=exw[:, 0:NSB * 4], in_=lst[:, 0:NSB * 4],
                                     func=AF.Exp, bias=0.0, scale=1.0)

                # vse: head-interleaved v * exp, plus denom cols
                vse = mpool.tile([128, NS * 132], bf16, tag="vse")
                vsev = vse[:].rearrange("p (n k) -> p n k", k=132)
                nc.vector.tensor_tensor(
                    out=vsev[:, 0:NSB, 0:128].rearrange("p n (w h) -> p n w h", h=4),
                    in0=gtb.rearrange("p (n k) -> p n k", k=256)[:, 0:NSB, 0:128]
                        .rearrange("p n (w h) -> p n w h", h=4),
                    in1=exw[:, 0:NSB * 4].rearrange("p (n h) -> p n h", h=4)
                        .unsqueeze(2).broadcast_to([128, NSB, 32, 4]),
                    op=OP.mult)
                nc.vector.tensor_copy(
                    out=vsev[:, 0:NSB, 128:132],
                    in_=exw[:, 0:NSB * 4].rearrange("p (n h) -> p n h", h=4))
                for n in range(NSB):
                    nc.tensor.matmul(
                        out=pout[:],
                        lhsT=mext[:, n * 128:(n + 1) * 128],
                        rhs=vse[:, n * 132:(n + 1) * 132],
                        start=(half == 0 and n == 0),
                        stop=(half == 1 and n == NSB - 1))

            den = spool.tile([128, 4], f32, tag="den")
            nc.scalar.activation(out=den[:], in_=pout[:, 128:132],
                                 func=AF.Copy, bias=1e-30, scale=1.0)
            rec = spool.tile([128, 4], f32, tag="rec")
            nc.vector.reciprocal(out=rec[:], in_=den[:])
            onorm = spool.tile([128, 128], f32, tag="onorm")
            nc.vector.tensor_tensor(
                out=onorm[:].rearrange("p (w h) -> p w h", h=4),
                in0=pout[:, 0:128].rearrange("p (w h) -> p w h", h=4),
                in1=rec[:].unsqueeze(1).broadcast_to([128, 32, 4]),
                op=OP.mult)
            nc.sync.dma_start(out_d[b * QB:(b + 1) * QB, :], onorm[0:QB, :])

    nc.compile()
    return nc


def _host_prep(h_obs, pos_obs, pos_query, src, dst, W1, b1, W2, b2, Wv, bv,
               log_sigma):
    import ml_dtypes
    bf = ml_dtypes.bfloat16

    src = np.asarray(src).astype(np.int64)
    dst = np.asarray(dst).astype(np.int64)
    h_obs = np.asarray(h_obs, dtype=np.float32)
    pos_obs = np.asarray(pos_obs, dtype=np.float32)
    pos_query = np.asarray(pos_query, dtype=np.float32)
    W1 = np.asarray(W1, dtype=np.float32)
    W2 = np.asarray(W2, dtype=np.float32)
    Wv = np.asarray(Wv, dtype=np.float32)
    b1 = np.asarray(b1, dtype=np.float32)
    b2 = np.asarray(b2, dtype=np.float32)
    bv = np.asarray(bv, dtype=np.float32)
    sigma = np.exp(np.float32(log_sigma)) + np.float32(1e-6)
    inv_sig2 = float(1.0 / (np.float64(sigma) ** 2))

    # per-core edge partition, then per (core, block, half) lists
    core_lists = []
    NSL = [[1, 1] for _ in range(NBLK)]
    edge_bounds = np.searchsorted(dst, np.arange(NCORES + 1) * QPC)
    for c in range(NCORES):
        e0, e1 = edge_bounds[c], edge_bounds[c + 1]
        dl = dst[e0:e1] - c * QPC
        sl = src[e0:e1]
        blocks = []
        blk_bounds = np.searchsorted(dl, np.arange(NBLK + 1) * QB)
        for b in range(NBLK):
            be0, be1 = blk_bounds[b], blk_bounds[b + 1]
            bsrc = sl[be0:be1]
            bdr = dl[be0:be1] - b * QB
            m = bsrc < HALF
            halves = []
            for hi, (hm, off) in enumerate(((m, 0), (~m, HALF))):
                s_h = (bsrc[hm] - off).astype(np.int16)
                d_h = bdr[hm].astype(np.float32)
                halves.append((s_h, d_h))
                NSL[b][hi] = max(NSL[b][hi], math.ceil(max(len(s_h), 1) / 128))
            blocks.append(halves)
        core_lists.append(blocks)
    NS = max(max(r) for r in NSL)
    NSP = NS * 128

    iota = np.broadcast_to(np.arange(128, dtype=np.float32), (128, 128))
    ident = np.eye(128, dtype=np.float32)
    w1qcb1 = np.concatenate([W1[0:3] + W1[3:6], b1[None, :]], 0).astype(np.float32)
    w1oc = np.zeros((4, 128), np.float32)
    w1oc[0:3] = W1[6:9] - W1[0:3]
    has_b1b2v = bool(np.any(b1) or np.any(b2) or np.any(bv))

    # head-interleaved Wv / bv: col w*4+h <- h*32+w
    wv_int = Wv.reshape(128, HEADS, HEAD_DIM).transpose(0, 2, 1).reshape(128, 128)
    bv_int = bv.reshape(HEADS, HEAD_DIM).T.reshape(128)

    # host-transposed h (bf16) per half: [128, HROWS]
    hTA = np.zeros((128, HROWS), bf)
    hTA[:, :HALF] = h_obs[:HALF].T.astype(bf)
    hTB = np.zeros((128, HROWS), bf)
    hTB[:, :N_O - HALF] = h_obs[HALF:].T.astype(bf)
    # pos packed [128, NTIL*4]: col 4t+i = pos[t*128+p, i]; col 4t+3 = 0 (co slot)
    def pos_pack(p):
        pp = np.zeros((HROWS, 4), np.float32)
        pp[:len(p), 0:3] = p
        return pp.reshape(NTIL, 128, 4).transpose(1, 0, 2).reshape(128, NTIL * 4)
    pcA = pos_pack(pos_obs[:HALF])
    pcB = pos_pack(pos_obs[HALF:])

    in_maps = []
    for c in range(NCORES):
        idxw = np.zeros((NBLK * 2, 128, NS * 8), np.int16)
        drel = np.full((NBLK * 2, 128, NS), -1.0, np.float32)
        for b in range(NBLK):
            for half in range(2):
                s_h, d_h = core_lists[c][b][half]
                n = len(s_h)
                ip = np.zeros(NSP, np.int16)
                ip[:n] = s_h
                w = ip.reshape(NSP // 16, 16).T  # [16, NS*8]
                idxw[b * 2 + half] = np.tile(w, (8, 1))
                dp = np.full(NSP, -1.0, np.float32)
                dp[:n] = d_h
                drel[b * 2 + half] = dp.reshape(NS, 128).T
        pqb = np.zeros((128, NBLK * 4), np.float32)
        qs = pos_query[c * QPC:(c + 1) * QPC]
        for b in range(NBLK):
            lo, hi = b * QB, min((b + 1) * QB, QPC)
            pqb[:hi - lo, b * 4:b * 4 + 3] = qs[lo:hi]
            pqb[:hi - lo, b * 4 + 3] = 1.0
        in_maps.append({
            "hTA": hTA, "hTB": hTB, "pcA": pcA, "pcB": pcB,
            "posq_blk": pqb,
            "idxw": idxw.reshape(NBLK * 2 * 128, NS * 8),
            "dstrel": drel.reshape(NBLK * 2 * 128, NS),
            "w1qcb1": w1qcb1,
            "w1ocf": w1oc,
            "sel124": np.eye(128, dtype=np.float32)[124:128],
            "w2": W2.astype(bf),
            "wv": wv_int.astype(bf),
            "b2rep": np.broadcast_to(b2, (128, 4)).copy().astype(np.float32),
            "bvrep": np.broadcast_to(bv_int, (128, 128)).copy().astype(np.float32),
            "iota_in": iota.astype(bf),
            "idf32": ident,
            "idbf": ident.astype(bf),
        })
    return NSL, inv_sig2, has_b1b2v, in_maps


def kernel(h_obs, pos_obs, pos_query, src, dst, W1, b1, W2, b2, Wv, bv,
           log_sigma, **_unused):
    import sys
    for p in ("/opt/trn_rl_repo", "/root/.axon_site/_ro/trn_rl_repo"):
        if p not in sys.path:
            sys.path.append(p)
    from concourse.bass_utils import run_bass_kernel_spmd

    NSL, inv_sig2, has_b1b2v, in_maps = _host_prep(
        h_obs, pos_obs, pos_query, src, dst, W1, b1, W2, b2, Wv, bv, log_sigma)

    key = (tuple(tuple(r) for r in NSL), round(inv_sig2, 9), has_b1b2v)
    if key not in _PROG_CACHE:
        _PROG_CACHE[key] = _build_program(NSL, inv_sig2, has_b1b2v)
    nc = _PROG_CACHE[key]

    res = run_bass_kernel_spmd(nc, in_maps, core_ids=list(range(NCORES)))
    outs = [np.asarray(r["out"])[:QPC] for r in res.results]
    full = np.concatenate(outs, axis=0).astype(np.float32)
    # un-interleave heads: col w*4+h -> h*32+w
    return np.ascontiguousarray(
        full.reshape(-1, HEAD_DIM, HEADS).transpose(0, 2, 1).reshape(-1, 128))


if __name__ == "__main__":
    pass
